# revision 61
# baseline (speedup 1.0000x reference)
"""Trainium2 Bass kernel for causal multi-head attention.

Problem: B=4, S=2048, D=512, H=8 heads (head_dim 64), causal mask.
  q = x @ Wq.T + bq ; k = x @ Wk.T + bk ; v = x @ Wv.T + bv
  att = softmax(mask(q k^T / sqrt(64))) @ v ; out = att @ Wo.T + bo

Sharding: 8 cores = (batch b in 0..3) x (head-group hg in 0..1, 4 heads each).
Each core computes its 4 heads' Q/K/V projections, attention, and a partial
out-projection (contribution of its head block). Host sums the two partials
per batch and adds bo. No collectives needed.

Device-side layout (all matmuls contract along SBUF partitions):
 - host feeds x transposed (xT [512, S]) and weights pre-transposed+packed so
   each weight tensor arrives in ONE wide DMA ([128, k*cols], et-tiles side
   by side); biases are folded in on the PSUM->SBUF path.
 - scores are computed TRANSPOSED (ST[k, q]) so exp(ST) is directly the
   stationary->moving operand for the attention*V matmul, and the softmax
   denominator falls out of that matmul via a ones-column interleaved into V.
 - the attention inner loop is software-pipelined at 1-k-tile granularity:
   each group owns a [128, 2*512] PSUM tile (both heads of the pair at a
   fixed 512-column offset, 2 banks); the pst pool double-buffers these so
   score matmuls of group g+1 overlap the (single, strided) exp of group g
   on the Scalar engine. AV matmuls trail three groups behind.
 - K is stored zero-padded to 128 contraction rows per head (row-masked
   tensor_scalar casts): plain 128-contraction score matmuls measure ~1.5x
   faster per column than the 64-row row-grouped tile mode.
 - causal triangle masking happens on the PE before exp: an identity-
   stationary matmul accumulates a -1e9 upper-triangle tile into the band
   score PSUM, so exp emits exact zeros and no mask multiply exists.
 - the NEXT q-block's projections and the PREVIOUS one's out-projection are
   sliced into small pieces and emitted between attention groups (software
   pipelining across phases), so the PE never head-of-line blocks on the
   normalization chain and the Scalar engine never starves.
 - engine balance: exp + K/Q PSUM casts were measured to pace the kernel, so
   exp stays on Scalar while K/Q/V casts, softmax normalization (32x32-block
   transpose + reciprocal trick), and out-proj PSUM->SBUF run on Vector;
   gpsimd does ONLY partition_broadcast + DMA (mixing op families on the
   Pool engine triggers ~6us GPSIMD library reloads per switch).
"""

import sys

import numpy as np

for _p in ("/opt/trn_rl_repo",):
    if _p not in sys.path:
        sys.path.insert(0, _p)

import ml_dtypes  # noqa: E402

import concourse.bass as bass  # noqa: E402
import concourse.tile as tile  # noqa: E402
from concourse import bacc, mybir  # noqa: E402

B, S, D, H = 4, 2048, 512, 8
HD = D // H  # 64
P = 128
HG = 4  # heads per core
DG = HG * HD  # 256 per-core head dims
QB = 512  # q-block (matmul moving free dim)
NQB = S // QB  # 4
NKT = S // P  # 16 k-tiles
KTQ = QB // P  # 4 k-tiles per q-block (diagonal band width)
NET = D // P  # 4 e-tiles (contraction tiles for projections)
VW = HG * (HD + 1)  # 260: V with an interleaved ones-column per head

F32 = mybir.dt.float32
BF16 = mybir.dt.bfloat16
NPBF16 = ml_dtypes.bfloat16

_BUILT = {}


def _build_nc(causal: bool):
    """Build (and bacc-compile) the SPMD single-core program."""
    nc = bacc.Bacc("TRN2", target_bir_lowering=False, debug=False, num_devices=8)

    xT_d = nc.dram_tensor("xT", [D, S], BF16, kind="ExternalInput").ap()
    wq_d = nc.dram_tensor("wq", [P, NET * DG], BF16, kind="ExternalInput").ap()
    bq_d = nc.dram_tensor("bqv", [P, 2], F32, kind="ExternalInput").ap()
    wk_d = nc.dram_tensor("wk", [P, NET * DG], BF16, kind="ExternalInput").ap()
    wv_d = nc.dram_tensor("wv", [P, NET * VW], BF16, kind="ExternalInput").ap()
    wvb_d = nc.dram_tensor("wvb", [1, VW], BF16, kind="ExternalInput").ap()
    wo_d = nc.dram_tensor("wo", [P, 2 * D], BF16, kind="ExternalInput").ap()
    if causal:
        # bmneg: 0 on/below the causal diagonal, -1e9 above; added into the
        # band score tiles via an identity-stationary matmul so masking
        # happens on the PE (before exp) instead of a DVE multiply after.
        bm_d = nc.dram_tensor("bm", [P, P], BF16, kind="ExternalInput").ap()
        id_d = nc.dram_tensor("ident", [P, P], BF16, kind="ExternalInput").ap()
    else:
        mt_d = nc.dram_tensor("mt", [HG, S, S], BF16, kind="ExternalInput").ap()
    out_d = nc.dram_tensor("out", [D, S], F32, kind="ExternalOutput").ap()

    EXP = mybir.ActivationFunctionType.Exp

    with tile.TileContext(nc) as tc:
        with (
            tc.tile_pool(name="consts", bufs=1) as consts,
            tc.tile_pool(name="work", bufs=4) as work,
            tc.tile_pool(name="attn", bufs=3) as attnp,
            tc.tile_pool(name="small", bufs=3) as small,
            tc.tile_pool(name="pmm", bufs=2, space="PSUM") as pmm,
            tc.tile_pool(name="pst", bufs=2, space="PSUM") as pst,
            tc.tile_pool(name="patt", bufs=2, space="PSUM") as patt,
        ):
            # ---- load persistent operands ----
            # First the operands the first projection matmuls need (wk + the
            # first q-block of x), then everything else, interleaved across
            # the sync and gpsimd DMA queues so descriptor issue overlaps.
            xts = [
                consts.tile([P, S], BF16, tag=f"xt{et}", name=f"xts{et}")
                for et in range(NET)
            ]
            # the first projection matmuls need wk + all four x first-blocks:
            # spread those across the three DMA-capable queues (sync,
            # scalar-hwdge, gpsimd-swdge) so issue overlaps.
            wk_sb = consts.tile([P, NET * DG], BF16, tag="wk")
            nc.sync.dma_start(out=wk_sb, in_=wk_d)
            for et, eng in zip(range(NET), (nc.gpsimd, nc.scalar, nc.gpsimd, nc.scalar)):
                eng.dma_start(
                    out=xts[et][:, 0:QB], in_=xT_d[et * P : (et + 1) * P, 0:QB]
                )
            wq_sb = consts.tile([P, NET * DG], BF16, tag="wq")
            nc.sync.dma_start(out=wq_sb, in_=wq_d)
            bq_sb = consts.tile([P, 2], F32, tag="bq")
            nc.sync.dma_start(out=bq_sb, in_=bq_d)
            wv_sb = consts.tile([P, NET * VW], BF16, tag="wv")
            nc.sync.dma_start(out=wv_sb, in_=wv_d)
            wvb = consts.tile([1, VW], BF16, tag="wvb")
            nc.sync.dma_start(out=wvb, in_=wvb_d)
            if causal:
                # q-block 0 is all band tiles: its first tri matmuls need
                # bm/ident almost immediately, so they go before x-rest/wo.
                bm = consts.tile([P, P], BF16, tag="bm")
                nc.scalar.dma_start(out=bm, in_=bm_d)
                ident = consts.tile([P, P], BF16, tag="ident")
                nc.scalar.dma_start(out=ident, in_=id_d)
            for et in range(NET):
                nc.gpsimd.dma_start(
                    out=xts[et][:, QB:S], in_=xT_d[et * P : (et + 1) * P, QB:S]
                )
            # broadcast the bias row once (after the x DMAs so it doesn't
            # head-of-line block their issue on the gpsimd queue); V-proj
            # folds it in during the PSUM->SBUF cast.
            bvb = consts.tile([P, VW], BF16, tag="bvb")
            nc.gpsimd.partition_broadcast(bvb, wvb)
            wo_sb = consts.tile([P, 2 * D], BF16, tag="wo")
            nc.sync.dma_start(out=wo_sb, in_=wo_d)

            def wk_t(et, dsl):
                return wk_sb[:, et * DG : (et + 1) * DG][:, dsl]

            def wq_t(et, dsl):
                return wq_sb[:, et * DG : (et + 1) * DG][:, dsl]

            def wv_t(et):
                return wv_sb[:, et * VW : (et + 1) * VW]

            def wo_t(j, esl):
                return wo_sb[:, j * D : (j + 1) * D][:, esl]

            # ---- Q/K/V projections, emitted per s-block so attention on the
            # first q-block can start while later blocks still project.
            # QT/KT transposed [dg, s]; V natural [s, (v|1) interleaved].
            QT = [consts.tile([P, S], BF16, tag=f"qt{i}", name=f"QT{i}") for i in range(2)]
            # K is stored zero-padded to full 128 contraction rows per head:
            # KZ[dgt][0] has head 2*dgt's dims in rows 0:64 and zeros in
            # 64:128 (vice versa for KZ[dgt][1]), so score matmuls run as
            # plain 128-contraction — the 64-row row-grouped tile mode
            # measures ~1.5x slower per column on hardware.
            KZ = [
                [
                    consts.tile([P, S], BF16, tag=f"kz{i}_{j}", name=f"KZ{i}{j}")
                    for j in range(2)
                ]
                for i in range(2)
            ]
            # per-partition 1/0 row masks: the K-proj PSUM->SBUF cast writes
            # each head's rows and the zero padding in ONE tensor_scalar
            # (full [64,2048] memsets here would serialize ~7us of DVE ahead
            # of the first score matmul).
            rmask = [
                consts.tile([P, 1], F32, tag=f"rm{j}", name=f"rmask{j}")
                for j in range(2)
            ]
            nc.vector.memset(rmask[0][0:HD, :], 1.0)
            nc.vector.memset(rmask[0][HD : 2 * HD, :], 0.0)
            nc.vector.memset(rmask[1][0:HD, :], 0.0)
            nc.vector.memset(rmask[1][HD : 2 * HD, :], 1.0)
            V = [
                consts.tile([P, VW], BF16, tag=f"v{st}", name=f"Vt{st}")
                for st in range(NKT)
            ]
            def kq_piece(sb, dgt, wt, dst, bias):
                """One K-or-Q projection piece: 4 matmuls + ACT cast."""
                ssl = slice(sb * QB, (sb + 1) * QB)
                dsl = slice(dgt * P, (dgt + 1) * P)
                ps = pmm.tile([P, QB], F32, tag="mm", name=f"pkq{sb}_{dgt}")
                for et in range(NET):
                    nc.tensor.matmul(
                        ps,
                        wt(et, dsl),
                        xts[et][:, ssl],
                        start=(et == 0),
                        stop=(et == NET - 1),
                    )
                if bias is None:
                    # K: row-masked casts into the zero-padded per-head tiles
                    nc.vector.tensor_scalar_mul(KZ[dgt][0][:, ssl], ps, rmask[0])
                    nc.vector.tensor_scalar_mul(KZ[dgt][1][:, ssl], ps, rmask[1])
                else:
                    nc.vector.tensor_scalar_add(dst[dgt][:, ssl], ps, bias)

            def v_piece(st):
                ksl = slice(st * P, (st + 1) * P)
                ps = pmm.tile([P, VW], F32, tag="mm", name=f"pv{st}")
                for et in range(NET):
                    nc.tensor.matmul(
                        ps,
                        xts[et][:, ksl],
                        wv_t(et),
                        start=(et == 0),
                        stop=(et == NET - 1),
                    )
                nc.vector.tensor_add(V[st], ps, bvb)

            def op_piece(qb, et):
                """One quarter of q-block qb's out-projection."""
                qsl = slice(qb * QB, (qb + 1) * QB)
                at = attn_done[qb]
                esl = slice(et * P, (et + 1) * P)
                ops = pmm.tile([P, QB], F32, tag="mm", name=f"pop{qb}_{et}")
                nc.tensor.matmul(ops, wo_t(0, esl), at[0], start=True, stop=False)
                nc.tensor.matmul(ops, wo_t(1, esl), at[1], start=False, stop=True)
                ost = work.tile([P, QB], F32, tag="ost", bufs=3)
                nc.vector.tensor_copy(ost, ops)
                nc.sync.dma_start(out=out_d[esl, qsl], in_=ost)

            def proj_pieces(sb):
                fs = []
                for dgt in range(2):
                    fs.append(lambda d=dgt, s=sb: kq_piece(s, d, wk_t, None, None))
                    fs.append(
                        lambda d=dgt, s=sb: kq_piece(
                            s, d, wq_t, QT, bq_sb[:, d : d + 1]
                        )
                    )
                for st in range(4 * sb, 4 * sb + 4):
                    fs.append(lambda s=st: v_piece(s))
                return fs

            # ---- attention + out-projection ----
            # Head PAIRS interleaved via tile_position: head 2*hp uses PE
            # rows 0..63, head 2*hp+1 rows 64..127.
            # Groups are 1 k-tile: a [128, 2w] PSUM score tile holds both
            # heads side by side; pst double-buffers them so scores(g+1)
            # run while exp(g) drains. AV matmuls trail 2 groups.
            # The NEXT q-block's projections and the PREVIOUS one's
            # out-projection are sliced into small pieces and emitted
            # between attention groups, so the PE queue always has score
            # work near the front and the Scalar engine never starves.

            # static per-head normalization staging tiles: rows 65..95 feed
            # the 32-row transpose but only row 64 is meaningful; memset the
            # junk rows once so the transpose never reads uninitialized SBUF.
            au_t = [
                consts.tile([HD + 32, QB], F32, tag=f"au{i}", name=f"au{i}")
                for i in range(2)
            ]
            for t in au_t:
                nc.vector.memset(t[HD : HD + 32, :], 0)
            # t2 is written only at every-32nd column by the reciprocal; the
            # transpose back reads the whole tile, so zero it once too.
            t2_t = [
                consts.tile([32, QB], F32, tag=f"t2{i}", name=f"t2s{i}")
                for i in range(2)
            ]
            for t in t2_t:
                nc.vector.memset(t, 0)

            attn_done = {}
            for f in proj_pieces(0):
                f()
            if not causal:
                # generic attention reads ALL k-tiles from its first q-block,
                # so projections cannot be deferred into the attention stream.
                for sb in range(1, NQB):
                    for f in proj_pieces(sb):
                        f()
            for qb in range(NQB):
                fillers = []
                if causal and qb + 1 < NQB:
                    fillers += proj_pieces(qb + 1)
                if qb > 0 and qb + 1 < NQB:
                    fillers += [
                        (lambda e=et, q=qb - 1: op_piece(q, e))
                        for et in range(NET)
                    ]
                gtotal = ((4 * qb + 4) if causal else NKT) * 2
                gdone = 0
                fi = 0
                qsl = slice(qb * QB, (qb + 1) * QB)
                attn_t = [
                    attnp.tile([P, QB], BF16, tag=f"attn{i}", name=f"attn{i}_{qb}")
                    for i in range(2)
                ]

                for hp in range(2):
                    hA, hB = 2 * hp, 2 * hp + 1
                    dgt = hp
                    rA, rB = slice(0, HD), slice(HD, 2 * HD)
                    attps = [
                        patt.tile([P, QB], F32, tag="att", name=f"att{qb}_{h}")
                        for h in (hA, hB)
                    ]
                    # (kt, q-offset within q-block, width, needs-triangle)
                    if causal:
                        groups = [(kt, 0, QB, False) for kt in range(qb * KTQ)]
                        b0 = qb * KTQ
                        for j in range(KTQ):
                            groups.append((b0 + j, j * P, QB - j * P, True))
                    else:
                        groups = [(kt, 0, QB, False) for kt in range(NKT)]
                    last_kt = groups[-1][0]
                    pendings = []

                    def flush(pend):
                        # head A at columns [0:w], head B at [QB:QB+w] (fixed
                        # 512 offset so each score matmul stays in one PSUM
                        # bank)
                        kt, w, exm = pend
                        for h, co, aps in ((hA, 0, attps[0]), (hB, QB, attps[1])):
                            nc.tensor.matmul(
                                aps[0 : HD + 1, QB - w : QB],
                                V[kt][:, h * (HD + 1) : (h + 1) * (HD + 1)],
                                exm[:, co : co + w],
                                start=(kt == 0),
                                stop=(kt == last_kt),
                            )

                    for gi, (kt, qo, w, tri) in enumerate(groups):
                        st = pst.tile([P, 2 * QB], F32, tag="st", name=f"st{qb}_{hp}_{kt}")
                        ksl = slice(kt * P, (kt + 1) * P)
                        qsub = slice(qb * QB + qo, (qb + 1) * QB)
                        nc.tensor.matmul(
                            st[:, 0:w], KZ[dgt][0][:, ksl], QT[dgt][:, qsub],
                            start=True, stop=True,
                        )
                        nc.tensor.matmul(
                            st[:, QB : QB + w], KZ[dgt][1][:, ksl], QT[dgt][:, qsub],
                            start=True, stop=True,
                        )
                        if tri:
                            # add -1e9 above the diagonal of the leading
                            # [128,128] triangle (identity.T @ bmneg = bmneg)
                            # directly in PSUM; exp then yields exact zeros.
                            # start=False accumulates onto the closed score
                            # group, which is fine on HW and in the interp.
                            for co in (0, QB):
                                nc.tensor.matmul(
                                    st[:, co : co + P], ident, bm,
                                    start=False, stop=True, skip_group_check=True,
                                )
                        exm = work.tile([P, 2 * QB], BF16, tag="ex", name=f"ex{qb}_{hp}_{kt}")
                        # scores are q.k / sqrt(64): fold 1/8 into the exp.
                        # One strided activation covers both heads' valid
                        # columns without touching the [w:QB] gap.
                        st3 = st.rearrange("p (t q) -> p t q", t=2)[:, :, 0:w]
                        ex3 = exm.rearrange("p (t q) -> p t q", t=2)[:, :, 0:w]
                        nc.scalar.activation(ex3, st3, EXP, scale=0.125)
                        if not causal:
                            for h, co in ((hA, 0), (hB, QB)):
                                mtile = work.tile([P, QB], BF16, tag="mt")
                                nc.sync.dma_start(
                                    out=mtile,
                                    in_=mt_d[h, kt * P : (kt + 1) * P, qsl],
                                )
                                nc.vector.tensor_mul(
                                    exm[:, co : co + w], exm[:, co : co + w], mtile
                                )
                        pendings.append((kt, w, exm))
                        if len(pendings) > 2:
                            flush(pendings.pop(0))
                        gdone += 1
                        while fi < len(fillers) and fi * gtotal < gdone * len(fillers):
                            fillers[fi]()
                            fi += 1
                    for pend in pendings:
                        flush(pend)
                    # for the final q-block, the previous block's
                    # out-projection runs right after the last AV flushes:
                    # it fills the PE while the final normalization chain
                    # (which gates this block's own out-projection) drains.
                    if qb == NQB - 1 and hp == 1:
                        for et in range(NET):
                            op_piece(qb - 1, et)
                    # normalize: rows 0..63 are sum(exp * v), row 64 is sum(exp)
                    # DVE reciprocal costs 6 cycles per FREE-dim element, so
                    # 1/sumexp on the [1,512] row is 3.3us. Instead transpose
                    # 32x32 blocks (row -> strided columns), reciprocal just
                    # the 16 real elements per partition (~0.1us), transpose
                    # back. Rows 65..95 of the PSUM tile are never written;
                    # their junk is copied around but only row 0 of t3 is read.
                    # phase-interleaved across the two heads: both PSUM
                    # copies first (frees the attps banks for the next head
                    # pair ASAP), then transposes/recips so the Pool
                    # broadcasts overlap the other head's DVE work, muls last.
                    heads = ((hA, attps[0], rA), (hB, attps[1], rB))
                    t3s, rbs = {}, {}
                    for h, aps, rsl in heads:
                        nc.vector.tensor_copy(
                            au_t[h % 2][0 : HD + 1, :], aps[0 : HD + 1, :]
                        )
                    for h, aps, rsl in heads:
                        au = au_t[h % 2]
                        t1 = small.tile([32, QB], F32, tag="t1")
                        nc.vector.transpose(t1, au[HD : HD + 32, :])
                        t2 = t2_t[h % 2]
                        nc.vector.reciprocal(
                            out=t2.rearrange("p (j c) -> p j c", c=32)[:, :, 0],
                            in_=t1.rearrange("p (j c) -> p j c", c=32)[:, :, 0],
                        )
                        t3 = small.tile([32, QB], F32, tag="t3")
                        nc.vector.transpose(t3, t2)
                        rb = small.tile([HD, QB], F32, tag="rb")
                        nc.gpsimd.partition_broadcast(rb, t3[0:1, :])
                        rbs[h] = rb
                    for h, aps, rsl in heads:
                        nc.vector.tensor_mul(
                            attn_t[dgt][rsl, :], au_t[h % 2][0:HD, :], rbs[h]
                        )
                while fi < len(fillers):
                    fillers[fi]()
                    fi += 1
                attn_done[qb] = attn_t
            for et in range(NET):
                op_piece(NQB - 1, et)

    nc.compile()
    return nc


def _get_nc(causal: bool):
    if causal not in _BUILT:
        _BUILT[causal] = _build_nc(causal)
    return _BUILT[causal]


def _band_mask():
    """[128, 128] additive tile: 0 where qi >= ki, -1e9 above the diagonal."""
    ki = np.arange(P)[:, None]
    qi = np.arange(P)[None, :]
    return np.where(qi >= ki, 0.0, -1e9).astype(np.float32).astype(NPBF16)


def _prep_core_inputs(x, mask, Wq, bq, Wk, Wv, bv, Wo, causal):
    """Build the 8 per-core input maps (bf16, pre-transposed, packed)."""
    bm = _band_mask()
    in_maps = []
    for c in range(8):
        b, hg = c // 2, c % 2
        h0, e0 = hg * HG, hg * DG
        xt = np.ascontiguousarray(x[b].T).astype(NPBF16)
        # [D, DG] weight, et-tiles packed side by side: [128, NET*DG]
        wq = Wq[e0 : e0 + DG, :].T.astype(NPBF16)
        wq_pack = np.ascontiguousarray(
            wq.reshape(NET, P, DG).transpose(1, 0, 2).reshape(P, NET * DG)
        )
        wk = Wk[e0 : e0 + DG, :].T.astype(NPBF16)
        wk_pack = np.ascontiguousarray(
            wk.reshape(NET, P, DG).transpose(1, 0, 2).reshape(P, NET * DG)
        )
        bqv = np.ascontiguousarray(
            bq[e0 : e0 + DG].reshape(2, P).T, dtype=np.float32
        )
        # V weights with ones-column interleaved per head for the softmax
        # denominator; bias row separate (broadcast on device).
        wv = np.zeros((D, VW), np.float32)
        wvb = np.zeros((1, VW), np.float32)
        for h in range(HG):
            eh = e0 + h * HD
            wv[:, h * (HD + 1) : h * (HD + 1) + HD] = Wv[eh : eh + HD, :].T
            wvb[0, h * (HD + 1) : h * (HD + 1) + HD] = bv[eh : eh + HD]
            wvb[0, h * (HD + 1) + HD] = 1.0
        wv_pack = np.ascontiguousarray(
            wv.reshape(NET, P, VW).transpose(1, 0, 2).reshape(P, NET * VW)
        ).astype(NPBF16)
        wo = Wo[:, e0 : e0 + DG].T.astype(NPBF16)  # [DG, D]
        wo_pack = np.ascontiguousarray(
            wo.reshape(2, P, D).transpose(1, 0, 2).reshape(P, 2 * D)
        )
        m = {
            "xT": xt,
            "wq": wq_pack,
            "bqv": bqv,
            "wk": wk_pack,
            "wv": wv_pack,
            "wvb": wvb.astype(NPBF16),
            "wo": wo_pack,
        }
        if causal:
            m["bm"] = bm
            m["ident"] = np.eye(P, dtype=np.float32).astype(NPBF16)
        else:
            # transposed multiplicative mask per local head: mt[h, k, q]
            mt = np.ascontiguousarray(
                mask[b, h0 : h0 + HG].transpose(0, 2, 1)
            ).astype(NPBF16)
            m["mt"] = mt
        in_maps.append(m)
    return in_maps


def kernel(**inputs):
    from concourse.bass_utils import run_bass_kernel_spmd

    x = np.asarray(inputs["x"], dtype=np.float32)
    mask = np.asarray(inputs["mask"])
    Wq = np.asarray(inputs["Wq"], dtype=np.float32)
    bq = np.asarray(inputs["bq"], dtype=np.float32)
    Wk = np.asarray(inputs["Wk"], dtype=np.float32)
    Wv = np.asarray(inputs["Wv"], dtype=np.float32)
    bv = np.asarray(inputs["bv"], dtype=np.float32)
    Wo = np.asarray(inputs["Wo"], dtype=np.float32)
    bo = np.asarray(inputs["bo"], dtype=np.float32)
    # bk is softmax-invariant (adds a per-query constant to all logits in a
    # row), so it is deliberately not used.

    causal = bool(
        (mask == np.tril(np.ones((S, S), dtype=bool))[None, None]).all()
    )

    nc = _get_nc(causal)
    in_maps = _prep_core_inputs(x, mask, Wq, bq, Wk, Wv, bv, Wo, causal)
    res = run_bass_kernel_spmd(nc, in_maps, core_ids=list(range(8)))
    out = np.empty((B, S, D), np.float32)
    for b in range(B):
        partial = res.results[2 * b]["out"] + res.results[2 * b + 1]["out"]
        out[b] = partial.T + bo[None, :]
    return out
